# revision 7
# baseline (speedup 1.0000x reference)
"""EncoderDecoderLSTM Trainium2 Bass kernel.

Strategy: 8-way data parallel over batch (B=512 -> 64/core), weights
replicated. Gates-on-partitions layout: z_T stored as [128, 16, 64]
(16 M-chunks of 128 gate rows x 64 batch), gate order [i, f, o, g] so
sigmoid(i|f|o) is one contiguous activation op. Hidden state stored as
H = h/2 so tanh(x) = 2*sigmoid(2x) - 1 folds into fused
scalar_tensor_tensor ops (single activation table -> no table reloads).
Biases folded into matmuls via ones-row augmentation.
"""
import os
import sys

sys.path.insert(0, "/opt/trn_rl_repo")

import numpy as np

NCORES = 8
B = 64            # batch per core
T = int(os.environ.get("LSTM_T", "256"))
HZ = 24
H = 512
KC = 4            # K chunks of hidden dim
MC = 16           # M chunks of gate dim

PERM = np.concatenate([
    np.arange(0, 512),       # i
    np.arange(512, 1024),    # f
    np.arange(1536, 2048),   # o
    np.arange(1024, 1536),   # g
])

_CACHE = {}


def _pack_whh(W, scale=2.0):
    # W [2048, K] -> [128, K/128, 2048]; [p,k,m] = scale*W[PERM[m], 128k+p]
    WT = (scale * W[PERM, :]).T
    K = WT.shape[0]
    return np.ascontiguousarray(
        WT.reshape(K // 128, 128, 2048).transpose(1, 0, 2)).astype(np.float32)


def _build(t_steps):
    import concourse.bacc as bacc
    import concourse.mybir as mybir
    import concourse.tile as tile

    f32 = mybir.dt.float32
    AF = mybir.ActivationFunctionType
    ALU = mybir.AluOpType

    nc = bacc.Bacc("TRN2", target_bir_lowering=False, debug=False)

    # ---- dram params
    dp = lambda name, shape: nc.declare_dram_parameter(name, list(shape), f32, isOutput=False)
    xT_d = dp("xT", (9, t_steps, B))
    wx_e0_d = dp("wx_e0", (9, 2048))
    w_e0_d = dp("w_e0", (128, KC, 2048))
    b_e1_d = dp("b_e1", (1, 2048))
    wx_e1_d = dp("wx_e1", (128, KC, 2048))
    w_e1_d = dp("w_e1", (128, KC, 2048))
    wx_d0_d = dp("wx_d0", (2, 2048))
    w_d0_d = dp("w_d0", (128, KC, 2048))
    b_d1_d = dp("b_d1", (2, 2048))
    wx_d1_d = dp("wx_d1", (128, KC, 2048))
    w_d1_d = dp("w_d1", (128, KC, 2048))
    w_out_d = dp("w_out", (128, KC))
    out_b_d = dp("out_b", (1, 1))
    preds_d = nc.declare_dram_parameter("preds", [HZ, B], f32, isOutput=True)
    h0st_d = nc.dram_tensor("h0st", [t_steps, 128, KC, B], f32)

    with tile.TileContext(nc) as tc:
        with (
            tc.tile_pool(name="persist", bufs=1) as pp,
            tc.tile_pool(name="gates", bufs=2) as gp,
            tc.tile_pool(name="zpool", bufs=2, space="PSUM") as zp,
            tc.tile_pool(name="predp", bufs=2, space="PSUM") as prp,
        ):
            H0 = pp.tile([128, KC, B], f32)
            C0 = pp.tile([128, KC, B], f32)
            H1 = pp.tile([128, KC, B], f32)
            C1 = pp.tile([128, KC, B], f32)
            predones = pp.tile([2, B], f32)
            ones = pp.tile([1, B], f32)
            preds_sb = pp.tile([1, HZ, B], f32)
            for st in (H0, C0, H1, C1):
                nc.vector.memset(st[:], 0.0)
            nc.vector.memset(predones[:], 1.0)
            nc.vector.memset(ones[:], 1.0)

            def cell(z, Hx, Cx):
                sifo = gp.tile([128, 12, B], f32, tag="sifo")
                nc.scalar.activation(sifo[:], z[:, 0:12, :], AF.Sigmoid)
                ug = gp.tile([128, 4, B], f32, tag="ug")
                nc.scalar.activation(ug[:], z[:, 12:16, :], AF.Sigmoid, scale=2.0)
                m1 = gp.tile([128, 4, B], f32, tag="m1")
                nc.vector.scalar_tensor_tensor(
                    m1[:], ug[:], 0.5, sifo[:, 0:4, :], ALU.subtract, ALU.mult)
                m2 = gp.tile([128, 4, B], f32, tag="m2")
                nc.vector.tensor_tensor(m2[:], sifo[:, 4:8, :], Cx[:], ALU.mult)
                nc.vector.tensor_tensor(Cx[:], m1[:], m2[:], ALU.add)
                uc = gp.tile([128, 4, B], f32, tag="uc")
                nc.scalar.activation(uc[:], Cx[:], AF.Sigmoid, scale=4.0)
                nc.vector.scalar_tensor_tensor(
                    Hx[:], uc[:], 0.5, sifo[:, 8:12, :], ALU.subtract, ALU.mult)

            ts = lambda m: slice(m * 128, (m + 1) * 128)

            # ================= phase L0 =================
            with tc.tile_pool(name="l0w", bufs=1) as wp:
                xT = wp.tile([9, t_steps, B], f32)
                wx_e0 = wp.tile([9, 2048], f32)
                w_e0 = wp.tile([128, KC, 2048], f32)
                nc.sync.dma_start(xT[:], xT_d[:])
                nc.sync.dma_start(wx_e0[:], wx_e0_d[:])
                nc.sync.dma_start(w_e0[:], w_e0_d[:])
                # decoder initial input = x[:, -1, 0] (raw feature 0)
                nc.vector.tensor_copy(predones[0:1, :], xT[0:1, t_steps - 1, :])

                for t in range(t_steps):
                    z = zp.tile([128, MC, B], f32, tag="z")
                    for m in range(MC):
                        nc.tensor.matmul(z[:, m, :], wx_e0[:, ts(m)], xT[:, t, :],
                                         start=True, stop=False)
                        for k in range(KC):
                            nc.tensor.matmul(z[:, m, :], w_e0[:, k, ts(m)],
                                             H0[:, k, :], start=False, stop=(k == KC - 1))
                    cell(z, H0, C0)
                    nc.sync.dma_start(h0st_d[t], H0[:])

            # ================= phase L1 =================
            with tc.tile_pool(name="l1w", bufs=1) as wp, \
                 tc.tile_pool(name="l1s", bufs=4) as sp:
                b_e1 = wp.tile([1, 2048], f32)
                wx_e1 = wp.tile([128, KC, 2048], f32)
                w_e1 = wp.tile([128, KC, 2048], f32)
                nc.sync.dma_start(b_e1[:], b_e1_d[:])
                nc.sync.dma_start(wx_e1[:], wx_e1_d[:])
                nc.sync.dma_start(w_e1[:], w_e1_d[:])

                for t in range(t_steps):
                    h0in = sp.tile([128, KC, B], f32, tag="h0in")
                    nc.sync.dma_start(h0in[:], h0st_d[t])
                    z = zp.tile([128, MC, B], f32, tag="z")
                    for m in range(MC):
                        nc.tensor.matmul(z[:, m, :], b_e1[0:1, ts(m)],
                                         ones[:], start=True, stop=False)
                        for k in range(KC):
                            nc.tensor.matmul(z[:, m, :], wx_e1[:, k, ts(m)],
                                             h0in[:, k, :], start=False, stop=False)
                        for k in range(KC):
                            nc.tensor.matmul(z[:, m, :], w_e1[:, k, ts(m)],
                                             H1[:, k, :], start=False, stop=(k == KC - 1))
                    cell(z, H1, C1)

            # ================= decoder =================
            with tc.tile_pool(name="dw", bufs=1) as wp:
                wx_d0 = wp.tile([2, 2048], f32)
                w_d0 = wp.tile([128, KC, 2048], f32)
                b_d1 = wp.tile([2, 2048], f32)
                wx_d1 = wp.tile([128, KC, 2048], f32)
                w_d1 = wp.tile([128, KC, 2048], f32)
                w_out = wp.tile([128, KC], f32)
                out_b = wp.tile([1, 1], f32)
                nc.sync.dma_start(wx_d0[:], wx_d0_d[:])
                nc.sync.dma_start(w_d0[:], w_d0_d[:])
                nc.sync.dma_start(b_d1[:], b_d1_d[:])
                nc.sync.dma_start(wx_d1[:], wx_d1_d[:])
                nc.sync.dma_start(w_d1[:], w_d1_d[:])
                nc.sync.dma_start(w_out[:], w_out_d[:])
                nc.sync.dma_start(out_b[:], out_b_d[:])

                for s in range(HZ):
                    z = zp.tile([128, MC, B], f32, tag="z")
                    for m in range(MC):
                        nc.tensor.matmul(z[:, m, :], wx_d0[:, ts(m)],
                                         predones[:], start=True, stop=False)
                        for k in range(KC):
                            nc.tensor.matmul(z[:, m, :], w_d0[:, k, ts(m)],
                                             H0[:, k, :], start=False, stop=(k == KC - 1))
                    cell(z, H0, C0)
                    z1 = zp.tile([128, MC, B], f32, tag="z")
                    for m in range(MC):
                        nc.tensor.matmul(z1[:, m, :], b_d1[:, ts(m)],
                                         predones[:], start=True, stop=False)
                        for k in range(KC):
                            nc.tensor.matmul(z1[:, m, :], wx_d1[:, k, ts(m)],
                                             H0[:, k, :], start=False, stop=False)
                        for k in range(KC):
                            nc.tensor.matmul(z1[:, m, :], w_d1[:, k, ts(m)],
                                             H1[:, k, :], start=False, stop=(k == KC - 1))
                    cell(z1, H1, C1)
                    pm = prp.tile([1, B], f32, tag="pm")
                    for k in range(KC):
                        nc.tensor.matmul(pm[:], w_out[:, k:k + 1], H1[:, k, :],
                                         start=(k == 0), stop=(k == KC - 1))
                    nc.vector.tensor_scalar(preds_sb[:, s, :], pm[:],
                                            out_b[0:1, 0:1], None, ALU.add)
                    nc.vector.tensor_copy(predones[0:1, :], preds_sb[:, s, :])

                nc.sync.dma_start(preds_d[:], preds_sb[:])

    nc.compile()
    return nc


def _get_nc(t_steps):
    key = ("v1", t_steps)
    if key not in _CACHE:
        _CACHE[key] = _build(t_steps)
    return _CACHE[key]


def _ensure_ntff_hook():
    # The image's `antenv` package lacks `axon_hooks`; register the ctypes
    # NTFF profiling hook from the boot module so trace=True can extract
    # exec_time_ns. Dev-profiling only (KERNEL_PROFILE=1).
    import types
    try:
        import antenv.axon_hooks  # noqa: F401
        return
    except ImportError:
        pass
    try:
        if "/root/.axon_site" not in sys.path:
            sys.path.insert(0, "/root/.axon_site")
        import trn_agent_boot.trn_boot as tb
        hook = tb._ntff_profile_via_ctypes("/opt/axon/libaxon_pjrt.so")
        mod = types.ModuleType("antenv.axon_hooks")
        mod.get_axon_ntff_profile_hook = lambda: hook
        mod.set_axon_ntff_profile_hook = lambda h: None
        sys.modules["antenv.axon_hooks"] = mod
    except Exception:
        pass


def kernel(**inputs):
    from concourse.bass_utils import run_bass_kernel_spmd

    x = np.asarray(inputs["x"], np.float32)
    Bfull, t_steps, F = x.shape
    assert Bfull == NCORES * B and F == 8

    shared = {
        "wx_e0": np.concatenate([
            inputs["e_Wih0"][PERM, :].T.astype(np.float32),
            (inputs["e_bih0"] + inputs["e_bhh0"])[PERM][None, :].astype(np.float32)]),
        "w_e0": _pack_whh(inputs["e_Whh0"]),
        "b_e1": (inputs["e_bih1"] + inputs["e_bhh1"])[PERM][None, :].astype(np.float32),
        "wx_e1": _pack_whh(inputs["e_Wih1"]),
        "w_e1": _pack_whh(inputs["e_Whh1"]),
        "wx_d0": np.stack([
            inputs["d_Wih0"][PERM, 0].astype(np.float32),
            (inputs["d_bih0"] + inputs["d_bhh0"])[PERM].astype(np.float32)]),
        "w_d0": _pack_whh(inputs["d_Whh0"]),
        "b_d1": np.stack([
            np.zeros(2048, np.float32),
            (inputs["d_bih1"] + inputs["d_bhh1"])[PERM].astype(np.float32)]),
        "wx_d1": _pack_whh(inputs["d_Wih1"]),
        "w_d1": _pack_whh(inputs["d_Whh1"]),
        "w_out": np.ascontiguousarray(
            (2.0 * inputs["out_W"][0]).reshape(KC, 128).T).astype(np.float32),
        "out_b": inputs["out_b"].reshape(1, 1).astype(np.float32),
    }
    shared = {k: np.ascontiguousarray(v) for k, v in shared.items()}

    in_maps = []
    for c in range(NCORES):
        xc = x[c * B:(c + 1) * B]                  # [64, T, 8]
        xT = np.empty((9, t_steps, B), np.float32)
        xT[0:8] = xc.transpose(2, 1, 0)
        xT[8] = 1.0
        in_maps.append({"xT": np.ascontiguousarray(xT), **shared})

    nc = _get_nc(t_steps)
    trace = os.environ.get("KERNEL_PROFILE", "0") == "1"
    if trace:
        _ensure_ntff_hook()
    res = run_bass_kernel_spmd(nc, in_maps, list(range(NCORES)), trace=trace)
    global LAST_EXEC_NS
    LAST_EXEC_NS = res.exec_time_ns

    out = np.empty((Bfull, HZ), np.float32)
    for c in range(NCORES):
        out[c * B:(c + 1) * B] = res.results[c]["preds"].T
    return out


LAST_EXEC_NS = None
